# revision 23
# baseline (speedup 1.0000x reference)
"""GQA causal attention with RoPE, distributed over 8 trn2 NeuronCores.

Sharding: 4-way data parallel over batch x 2-way tensor parallel over heads.
Core c = 2*b + t handles batch b with query heads [t*8, (t+1)*8) and KV heads
[t*2, (t+1)*2).  Each core computes a row-sharded out_proj partial; the pair
partials are summed on the host during unsharding.

On-chip algorithm (per core, bf16 matmuls / fp32 softmax):
  1. QKV projections from host-pretransposed xT (feature-major).
  2. Each projection chunk is staged PSUM->SBUF (bf16) on the otherwise-idle
     ScalarE; RoPE then runs all-bf16 on the DVE (2x perf mode) and the
     PSUM bank frees early.  Chunk m's transposes sit behind chunk m+1's
     projection matmuls in PE program order, so the PE never waits on the
     RoPE chain in steady state.
  3. Scores computed TRANSPOSED (scoresT[k_tok, q_tok]) so no probs
     transpose is needed: exp on ScalarE, column sums accumulated on DVE,
     summed across partitions by a ones-matmul, AV matmul consumes probsT
     directly, normalization happens once on the attention output.
  4. Causality: blocks with ki > qi are never computed; the diagonal block
     is masked with a precomputed upper-triangular 0/1 mask after exp.
  5. out_proj from feature-major attnT with wo as the moving operand,
     fine-grained (one 512-col PSUM bank per block, k-inner accumulation).

Pool layout is managed explicitly: phase B's SBUF pools are allocated
before phase A's so they do not overlap (no release-barrier between the
phases), and one shared 4-slot [P,512] PSUM pool serves the score tiles,
the ones-matmul sum tiles and the out_proj tiles, so phases flow into
each other without PSUM bank waits.
"""

import math
import sys

sys.path.insert(0, "/opt/trn_rl_repo")

import ml_dtypes
import numpy as np

import concourse.bacc as bacc
import concourse.mybir as mybir
import concourse.tile as tile
from concourse.bass import _add_dep_helper
from concourse.bass_utils import run_bass_kernel_spmd
from concourse.masks import make_upper_triangular

B, S, HID = 4, 1024, 2048
H, KV, D = 16, 4, 128
P = 128
TP = 2                  # tensor-parallel ways (head split)
HL = H // TP            # 8 query heads per core
KVL = KV // TP          # 2 kv heads per core
QD = HL * D             # 1024
KD = KVL * D            # 256
SC = S // P             # 8 token chunks
KC = HID // P           # 16 hidden chunks
NCORES = 8
BF = mybir.dt.bfloat16
F32 = mybir.dt.float32
Exp = mybir.ActivationFunctionType.Exp

_NC_CACHE = {}


def _ensure_ntff_hook():
    """The agent image's antenv lacks axon_hooks, so bass_utils' trace=True
    path can't find the NTFF profile hook trn_boot would have registered.
    Recreate the module and register the ctypes-based hook ourselves."""
    try:
        from antenv.axon_hooks import get_axon_ntff_profile_hook  # noqa: F401
        return
    except ImportError:
        pass
    import types

    import antenv

    mod = types.ModuleType("antenv.axon_hooks")
    _state = {"hook": None}
    mod.set_axon_ntff_profile_hook = lambda h: _state.__setitem__("hook", h)
    mod.get_axon_ntff_profile_hook = lambda: _state["hook"]
    sys.modules["antenv.axon_hooks"] = mod
    antenv.axon_hooks = mod
    try:
        from trn_agent_boot.trn_boot import _ntff_profile_via_ctypes

        hook = _ntff_profile_via_ctypes("/opt/axon/libaxon_pjrt.so")
        if hook is not None:
            mod.set_axon_ntff_profile_hook(hook)
    except Exception as e:  # pragma: no cover
        print(f"NTFF hook registration failed: {e}", file=sys.stderr)


def _pieces(start, end, step=512):
    """Split [start, end) into spans of at most `step`, aligned so no span
    crosses a `step` boundary (PSUM: one bank per matmul)."""
    out = []
    a = start
    while a < end:
        b = min((a // step + 1) * step, end)
        out.append((a, b))
        a = b
    return out


def build_nc():
    nc = bacc.Bacc("TRN2", target_bir_lowering=False, debug=False,
                   num_devices=NCORES)

    QKVD = QD + 2 * KD          # 1536 = q 1024 | k 256 | v 256
    xT = nc.declare_dram_parameter("xT", [HID, S], BF, isOutput=False)
    wqkv = nc.declare_dram_parameter("wqkv", [HID, QKVD], BF, isOutput=False)
    wo = nc.declare_dram_parameter("wo", [QD, HID], BF, isOutput=False)
    cos_t = nc.declare_dram_parameter("cos_t", [S, D], BF, isOutput=False)
    sin_t = nc.declare_dram_parameter("sin_t", [S, D], BF, isOutput=False)
    out = nc.declare_dram_parameter("out", [S, HID], BF, isOutput=True)

    with tile.TileContext(nc) as tc:
        # ---- persistent pools (allocated first: fixed addresses) ----
        cpool = tc.alloc_tile_pool(name="consts", bufs=1)
        wpool = tc.alloc_tile_pool(name="wpool", bufs=1)
        qkvpool = tc.alloc_tile_pool(name="qkvpool", bufs=1)
        # phase B's SBUF pool allocated BEFORE phase A's pools so their
        # address ranges are disjoint: no release barrier between A and B.
        battn = tc.alloc_tile_pool(name="battn", bufs=2)

        utmask = cpool.tile([P, P], BF)
        ones_mat = cpool.tile([P, P], BF)

        sb_wo = wpool.tile([P, HL, HID], BF)

        sb_qT = qkvpool.tile([P, HL, S], BF)      # feature-major q
        sb_kT = qkvpool.tile([P, KVL, S], BF)     # feature-major k
        sb_v = qkvpool.tile([P, SC, KD], BF)      # token-major v
        sb_attnT = qkvpool.tile([P, HL, S], BF)   # feature-major attn out

        # ---------------- Phase A: projections + RoPE -----------------
        projpool = tc.alloc_tile_pool(name="proj", bufs=1)
        ropepool = tc.alloc_tile_pool(name="rope", bufs=2)
        ps_q = tc.alloc_tile_pool(name="ps_q", bufs=2, space="PSUM")

        # chunk 0 first (time-to-first-matmul), cos/sin next (needed by the
        # first RoPE at ~16us), then the rest streamed per-chunk
        sb_xT = projpool.tile([P, KC, S], BF)
        sb_wqkv = projpool.tile([P, KC, QKVD], BF)
        xT_r = xT.rearrange("(c p) s -> p c s", p=P)
        wqkv_r = wqkv.rearrange("(c p) n -> p c n", p=P)
        # chunk 0 is split fine so the very first matmul only waits on
        # ~200 KB (first n-block of wqkv + first token columns of xT)
        nc.sync.dma_start(out=sb_wqkv[:, 0, 0:512], in_=wqkv_r[:, 0, 0:512])
        nc.sync.dma_start(out=sb_xT[:, 0, 0:256], in_=xT_r[:, 0, 0:256])
        nc.sync.dma_start(out=sb_wqkv[:, 0, 512:1536], in_=wqkv_r[:, 0, 512:1536])
        nc.sync.dma_start(out=sb_xT[:, 0, 256:1024], in_=xT_r[:, 0, 256:1024])
        nc.sync.dma_start(out=sb_wqkv[:, 1, :], in_=wqkv_r[:, 1, :])
        nc.sync.dma_start(out=sb_xT[:, 1, :], in_=xT_r[:, 1, :])
        sb_ck = projpool.tile([P, SC, D], BF)
        nc.sync.dma_start(
            out=sb_ck[:, :, :],
            in_=cos_t.rearrange("(m p) d -> p m d", p=P),
        )
        sb_sk = projpool.tile([P, SC, D], BF)
        nc.sync.dma_start(
            out=sb_sk[:, :, :],
            in_=sin_t.rearrange("(m p) d -> p m d", p=P),
        )
        for c in range(2, KC):
            nc.sync.dma_start(out=sb_wqkv[:, c, :], in_=wqkv_r[:, c, :])
            nc.sync.dma_start(out=sb_xT[:, c, :], in_=xT_r[:, c, :])
        # wo is only needed in phase C: delay its (4 MB) load until the
        # input streaming has drained
        wo_dma = nc.sync.dma_start(
            out=sb_wo[:, :, :],
            in_=wo.rearrange("(c p) n -> p c n", p=P),
        )

        # mask/ones builders issued after the DMA starts so the sync engine
        # kicks off the input stream first (they are not needed until B)
        make_upper_triangular(nc, utmask[:, :], val=1.0, diag=True)
        nc.vector.memset(ones_mat[:, :], 1.0)

        HALF = D // 2

        def rope_block(sb_src, lo, nh, m):
            """RoPE `nh` consecutive heads of the staged bf16 chunk (cols
            [lo, lo+nh*D)) in one batched op per step, via free-dim-broadcast
            cos/sin APs.  All-bf16 so the DVE runs in 2x mode.  Returns a
            bf16 SBUF tile [P, nh*D]."""
            t1 = ropepool.tile([P, nh, D], BF, tag="t1")
            ro = ropepool.tile([P, nh * D], BF, tag="ro")
            src = sb_src[:, lo:lo + nh * D].rearrange("p (h d) -> p h d", h=nh)
            sin_lo = sb_sk[:, m:m + 1, 0:HALF].broadcast_to([P, nh, HALF])
            sin_hi = sb_sk[:, m:m + 1, HALF:D].broadcast_to([P, nh, HALF])
            cos_b = sb_ck[:, m:m + 1, :].broadcast_to([P, nh, D])
            # rot_half * sin (sin table pre-negated on first half)
            nc.vector.tensor_mul(t1[:, :, 0:HALF], src[:, :, HALF:D], sin_lo)
            nc.vector.tensor_mul(t1[:, :, HALF:D], src[:, :, 0:HALF], sin_hi)
            ror = ro[:, :].rearrange("p (h d) -> p h d", h=nh)
            # ro = src*cos + t1
            nc.vector.tensor_mul(ror, src, cos_b)
            nc.vector.tensor_add(ror, ror, t1[:, :, :])
            return ro


        def proj_mms(pqkv, m, k, nblocks=(0, 1, 2)):
            st, sp = (k == 0), (k == KC - 1)
            lhsT = sb_xT[:, k, m * P:(m + 1) * P]
            for n in nblocks:
                mm = nc.tensor.matmul(
                    pqkv[:, n * 512:(n + 1) * 512], lhsT,
                    sb_wqkv[:, k, n * 512:(n + 1) * 512],
                    start=st, stop=sp)
            return mm

        def stage_m(pqkv, m):
            # Stage the chunk PSUM->SBUF bf16 on ScalarE (k-part first so the
            # k RoPE starts earliest).  This frees the PSUM bank ~1.4us after
            # the matmuls instead of after the whole RoPE chain.
            sb_qk = ropepool.tile([P, QD + KD], BF, tag="qk")
            nc.scalar.copy(sb_qk[:, QD:QD + KD], pqkv[:, QD:QD + KD])
            nc.scalar.copy(sb_v[:, m, :], pqkv[:, QD + KD:QD + 2 * KD])
            nc.scalar.copy(sb_qk[:, 0:QD], pqkv[:, 0:QD])
            return sb_qk

        def rope_T_m(sb_qk, m):
            # RoPE on DVE (2x bf16), then the hardware XBAR transpose on an
            # otherwise-idle DMA engine (8 x 16x128 tiles / 128-col group at
            # 14ns per tile) -- the PE does no transpose work at all.
            ms = slice(m * P, (m + 1) * P)
            k_ro = rope_block(sb_qk, QD, KVL, m)
            nc.sync.dma_start_transpose(out=sb_kT[:, :, ms], in_=k_ro[:, :])
            q_ro1 = rope_block(sb_qk, 0, 4, m)
            nc.sync.dma_start_transpose(out=sb_qT[:, 0:4, ms], in_=q_ro1[:, :])
            q_ro2 = rope_block(sb_qk, 4 * D, 4, m)
            nc.sync.dma_start_transpose(out=sb_qT[:, 4:8, ms], in_=q_ro2[:, :])

        def finish_m(pqkv, m):
            rope_T_m(stage_m(pqkv, m), m)

        # m=0, m=1 and m=2's q-part all share each arriving k-chunk during
        # the DMA ramp (the ramp is bandwidth-bound, so every spare PSUM
        # bank of accumulation is free compute).  m2's q-part lives in its
        # own 2-bank tile; its kv-part follows once m0's slot frees.  From
        # m=3 on, chunk m's projection matmuls are issued BEFORE chunk
        # m-1's stage+RoPE, so the PE never waits on the RoPE chain.
        ps_q2 = tc.alloc_tile_pool(name="ps_q2", bufs=1, space="PSUM")
        pqkv0 = ps_q.tile([P, QKVD], F32, tag="pqkv")
        pqkv1 = ps_q.tile([P, QKVD], F32, tag="pqkv")
        pq2 = ps_q2.tile([P, QD], F32, tag="pq2")
        for k in range(8):
            proj_mms(pqkv0, 0, k)
            proj_mms(pqkv1, 1, k)
            proj_mms(pq2, 2, k, nblocks=(0, 1))
        for k in range(8, KC):
            proj_mms(pqkv0, 0, k)
            proj_mms(pq2, 2, k, nblocks=(0, 1))
        # m0's stage+RoPE runs during m1's back half, so pqkv0 is free the
        # moment m2's kv matmuls want it
        finish_m(pqkv0, 0)
        for k in range(8, KC):
            proj_mms(pqkv1, 1, k)
        pqkv2 = ps_q.tile([P, QKVD], F32, tag="pqkv")
        for k in range(KC):
            mm = proj_mms(pqkv2, 2, k, nblocks=(2,))
        # release the wo load only once the input streaming has drained
        _add_dep_helper(wo_dma.ins, mm.ins,
                        reason="delay wo load past input ramp")
        finish_m(pqkv1, 1)

        def stage_m2():
            sb_qk = ropepool.tile([P, QD + KD], BF, tag="qk")
            nc.scalar.copy(sb_qk[:, QD:QD + KD], pqkv2[:, QD:QD + KD])
            nc.scalar.copy(sb_v[:, 2, :], pqkv2[:, QD + KD:QD + 2 * KD])
            nc.scalar.copy(sb_qk[:, 0:QD], pq2[:, :])
            return sb_qk

        prev = None
        for m in range(3, SC):
            pqkv = ps_q.tile([P, QKVD], F32, tag="pqkv")
            for k in range(KC):
                proj_mms(pqkv, m, k)
            if m == 3:
                rope_T_m(stage_m2(), 2)
            else:
                finish_m(*prev)
            prev = (pqkv, m)

        # Last chunk: stage it, free the projection PSUM, and immediately
        # issue the first heads' early score pieces (they only touch token
        # chunks 0-3) so the PE and ScalarE ramp into phase B while the DVE
        # runs the last RoPE chain.
        sb_qk_last = stage_m(*prev)
        ps_q2.release()
        ps_q.release()
        # Shared 4-slot [P,512] fp32 PSUM pool for score tiles, ones-matmul
        # sum tiles AND phase C's out_proj tiles: slot rotation replaces
        # inter-phase pool barriers, and 4 slots are deep enough that the
        # PE never waits for an exp/reciprocal to free a bank.
        ps_small = tc.alloc_tile_pool(name="ps_small", bufs=4, space="PSUM")
        ps_av = tc.alloc_tile_pool(name="ps_av", bufs=2, space="PSUM")

        exp_scale = float(1 / math.sqrt(D))
        head_tiles = {}

        def get_head_tiles(h):
            if h not in head_tiles:
                probsT = battn.tile([P, SC, S], BF, tag="probsT",
                                    name=f"probsT{h}")
                acc = battn.tile([P, S], BF, tag="acc", name=f"acc{h}")
                head_tiles[h] = (probsT, acc)
            return head_tiles[h]

        early_done = set()
        for h in (0, 1):
            probsT, _ = get_head_tiles(h)
            for ki in range(4):
                a = ki * P
                psc = ps_small.tile([P, 512], F32, tag="ps", name="psce")
                nc.tensor.matmul(psc[:, 0:512 - a], sb_kT[:, 0, a:a + P],
                                 sb_qT[:, h, a:512], start=True, stop=True)
                nc.scalar.activation(probsT[:, ki, a:512], psc[:, 0:512 - a],
                                     Exp, scale=exp_scale)
                early_done.add((h, ki, a))

        rope_T_m(sb_qk_last, prev[1])
        ropepool.release()
        projpool.release()

        # ---------------- Phase B: causal attention -------------------

        def make_head(h, g, probsT, acc, pav):
            def av(ki):
                st, sp = (ki == 0), (ki == SC - 1)
                for (a, b) in _pieces(ki * P, S):
                    nc.tensor.matmul(pav[:, a:b],
                                     sb_v[:, ki, g * D:(g + 1) * D],
                                     probsT[:, ki, a:b],
                                     start=st, stop=sp)

            def finalize():
                av(SC - 1)
                # ones-matrix matmul = column sums already broadcast across
                # all partitions, straight into PSUM
                rbc = battn.tile([P, S], F32, tag="rbc", bufs=1)
                for (a, b) in _pieces(0, S):
                    psbc = ps_small.tile([P, 512], F32, tag="ps")
                    nc.tensor.matmul(psbc[:, 0:b - a], ones_mat[:, :],
                                     acc[:, a:b], start=True, stop=True)
                    nc.vector.reciprocal_approx_fast(rbc[:, a:b],
                                                     psbc[:, 0:b - a])
                nc.vector.tensor_mul(sb_attnT[:, h, :], pav[:, :], rbc[:, :])

            return av, finalize

        pending = [None]
        for h in range(HL):
            g = h // (HL // KVL)
            probsT, acc = get_head_tiles(h)
            pav = ps_av.tile([P, S], F32, tag="pav")
            av, finalize = make_head(h, g, probsT, acc, pav)

            for ki in range(SC):
                q0 = ki * P
                kslice = slice(q0, q0 + P)
                for (a, b) in _pieces(q0, S):
                    if (h, ki, a) in early_done:
                        continue
                    psc = ps_small.tile([P, 512], F32, tag="ps")
                    nc.tensor.matmul(psc[:, 0:b - a],
                                     sb_kT[:, g, kslice],
                                     sb_qT[:, h, a:b],
                                     start=True, stop=True)
                    nc.scalar.activation(probsT[:, ki, a:b],
                                         psc[:, 0:b - a], Exp,
                                         scale=exp_scale)
                # mask strictly-below-diagonal of the diag block, on the
                # otherwise-idle GpSimd (the DVE is phase B's bottleneck)
                nc.gpsimd.tensor_mul(probsT[:, ki, q0:q0 + P],
                                     probsT[:, ki, q0:q0 + P],
                                     utmask[:, :])
                # accumulate the column sums on DVE (2x bf16): one tile add
                # per ki instead of 12 ones-matmuls per head on the PE
                if ki == 0:
                    nc.vector.tensor_copy(acc[:, :], probsT[:, 0, :])
                else:
                    nc.vector.tensor_add(acc[:, q0:], acc[:, q0:],
                                         probsT[:, ki, q0:])
                if ki >= 1:
                    av(ki - 1)
                # previous head's ~4 us tail chain runs inside this head's
                # compute instead of stalling the PE
                if ki == 3 and pending[0] is not None:
                    pending[0]()
                    pending[0] = None
            pending[0] = finalize
        pending[0]()

        # ---------------- Phase C: out projection ---------------------
        # Fine-grained: one 512-col PSUM slot per n-block with k-inner
        # accumulation.  The slots come from ps_small, so the first blocks
        # start while phase B's last head drains; the tail after the final
        # matmul is a single [P,512] copy + one DMA.
        ypool = tc.alloc_tile_pool(name="ysb", bufs=2)
        for m in range(SC):
            ms = slice(m * P, (m + 1) * P)
            ysb = ypool.tile([P, HID], BF, tag="ysb")
            for nb in range(HID // 512):
                nsl = slice(nb * 512, (nb + 1) * 512)
                py = ps_small.tile([P, 512], F32, tag="ps")
                for k in range(HL):
                    nc.tensor.matmul(py[:, :],
                                     sb_attnT[:, k, ms],
                                     sb_wo[:, k, nsl],
                                     start=(k == 0), stop=(k == HL - 1))
                # both copy engines are idle in phase C: alternate
                if nb % 2 == 0:
                    nc.scalar.copy(ysb[:, nsl], py[:, :])
                else:
                    nc.vector.tensor_copy(ysb[:, nsl], py[:, :])
                    # store per 1024-col pair so the final tail is only one
                    # [P,512] copy + a 256 KB DMA
                    nc.sync.dma_start(out=out[ms, nb * 512 - 512:nb * 512 + 512],
                                      in_=ysb[:, nb * 512 - 512:nb * 512 + 512])

        ypool.release()
        ps_av.release()
        ps_small.release()
        battn.release()
        qkvpool.release()
        wpool.release()
        cpool.release()

    nc.compile()
    return nc


def _get_nc():
    if "nc" not in _NC_CACHE:
        _NC_CACHE["nc"] = build_nc()
    return _NC_CACHE["nc"]


def _make_in_maps(x, cos, sin, wq, wk, wv, wo):
    bf = ml_dtypes.bfloat16
    HALF = D // 2
    sin_rot = np.concatenate([-sin[:, :HALF], sin[:, HALF:]], axis=1)
    cos_t = cos.astype(bf)
    sin_t = sin_rot.astype(bf)
    in_maps = []
    for core in range(NCORES):
        b, t = divmod(core, TP)
        wqkv = np.concatenate([
            wq[:, t * QD:(t + 1) * QD],
            wk[:, t * KD:(t + 1) * KD],
            wv[:, t * KD:(t + 1) * KD],
        ], axis=1)
        in_maps.append({
            "xT": np.ascontiguousarray(x[b].T).astype(bf),
            "wqkv": np.ascontiguousarray(wqkv).astype(bf),
            "wo": np.ascontiguousarray(wo[t * QD:(t + 1) * QD, :]).astype(bf),
            "cos_t": cos_t, "sin_t": sin_t,
        })
    return in_maps


def run(inputs, trace=False):
    if trace:
        _ensure_ntff_hook()
    nc = _get_nc()
    in_maps = _make_in_maps(
        np.asarray(inputs["x"], np.float32),
        np.asarray(inputs["cos"], np.float32),
        np.asarray(inputs["sin"], np.float32),
        np.asarray(inputs["wq"], np.float32),
        np.asarray(inputs["wk"], np.float32),
        np.asarray(inputs["wv"], np.float32),
        np.asarray(inputs["wo"], np.float32),
    )
    res = run_bass_kernel_spmd(nc, in_maps, list(range(NCORES)), trace=trace)
    outs = [np.asarray(r["out"]).astype(np.float32) for r in res.results]
    y = np.stack([outs[TP * b] + outs[TP * b + 1] for b in range(B)])
    return y, res


def kernel(**inputs):
    y, _ = run(inputs, trace=False)
    return y
